# revision 1
# baseline (speedup 1.0000x reference)
# CMPN encoder Bass kernel for 8-core TRN2, v2.
#
# Sharding: atoms by molecule (6400/core + atom0 replicated + pad = 6528);
# bonds reassigned so owner(b) = owner(atom b2a[b]) -> all b2a gathers local.
# Per round ONE AllToAll exchanges the bond-message rows each consumer needs
# (deduped union of its a2b and b2revb draws), delivered into T_union.
# Consumption uses int16 dma_gather with two 32768-row windows; slots whose
# source falls outside a window point at a zero row (messages are >= 0, so a
# zero row is an identity for both the sum and the max of the aggregation,
# and contributes nothing to the 2-part rev subtraction).
# Tables are bf16 with rows padded to 384 cols (zeros) so dma_gather's
# 256B-multiple elem_size holds and matmul K-chunks are exactly 3x128.
import numpy as np
import ml_dtypes
import concourse.bass as bass
import concourse.bacc as bacc
import concourse.tile as tile
from concourse import mybir
from concourse.library_config import mlp
from concourse.masks import make_identity

F32 = mybir.dt.float32
BF16 = mybir.dt.bfloat16
I16 = mybir.dt.int16
AF = mybir.ActivationFunctionType
ALU = mybir.AluOpType

H = 300
HP = 384
B = 1024
A = 50
NA = 51201
NB = 102401
AFD = 133
BFD = 147
N_CORES = 8
NMOL = B // N_CORES          # 128
ATOMS = NMOL * A             # 6400 real atoms/core
AT = 51                      # atom tiles: 6400 + atom0 + pad = 6528
DEPTH = 4
KCH = [(0, 128), (128, 256), (256, 300)]
WIN = 32768


# ======================= host-side routing prep ==========================

def _wrap_idx(vals):
    """int16 index list -> [128, ceil(n/16)] wrapped/replicated layout."""
    v = np.asarray(vals, np.int64)
    n = len(v)
    cols = -(-n // 16)
    arr = np.zeros((16, cols), np.int16)
    pad = np.zeros(cols * 16, np.int64)
    pad[:n] = v
    assert pad.max() < 32768 and pad.min() >= -32768
    arr = pad.reshape(cols, 16).T.astype(np.int16)
    return np.tile(arr, (8, 1))


def build_routing(a2b, b2a, b2revb):
    """All static routing. Returns per-core dicts + global sizes."""
    rt = {}
    owner_b = np.where(b2a >= 1, (b2a - 1) // 6400, np.arange(NB) % N_CORES)
    bonds = [np.where(owner_b == k)[0] for k in range(N_CORES)]
    nbond_max = max(len(x) for x in bonds)
    BT = -(-nbond_max // 128)
    NBND = BT * 128

    # consumer draw lists (atom agg for 6528-atom layout incl atom0 + pads)
    atom_ids = []
    for k in range(N_CORES):
        ids = np.concatenate([np.arange(6400 * k + 1, 6400 * (k + 1) + 1),
                              [0], np.zeros(127, np.int64)])
        atom_ids.append(ids)
    # agg slot draws: for pad atoms use bond draw "-1" => dummy
    agg_draws = []
    for k in range(N_CORES):
        d = np.full((AT * 128, 6), -1, np.int64)
        ids = atom_ids[k]
        real = np.concatenate([np.arange(6400), [6400]])  # local rows w/ draws
        d[real] = a2b[ids[real]]
        agg_draws.append(d)

    rev_draws = []
    for k in range(N_CORES):
        d = np.full(NBND, -1, np.int64)
        d[:len(bonds[k])] = b2revb[bonds[k]]
        rev_draws.append(d)

    # per-consumer union of needed bond rows, partitioned by owner
    I = [[None] * N_CORES for _ in range(N_CORES)]  # I[j][k]: global bond ids
    for k in range(N_CORES):
        u = np.unique(np.concatenate([agg_draws[k].ravel(), rev_draws[k]]))
        u = u[u >= 0]
        for j in range(N_CORES):
            I[j][k] = u[owner_b[u] == j]
    S = max(len(I[j][k]) for j in range(N_CORES) for k in range(N_CORES))
    S_pad = -(-S // 128) * 128
    NT = 2 + 8 * S_pad          # T_union rows: [z0 | recv 8*S_pad | z1]
    W2B = NT - WIN              # base row of window 2
    assert NT - 1 - W2B <= 32767 and WIN - 1 <= 32767

    # position of global bond id in consumer k's T_union (1-based after z0)
    pos = []
    for k in range(N_CORES):
        p = {}
        for j in range(N_CORES):
            for i, b in enumerate(I[j][k]):
                p[b] = 1 + j * S_pad + i
        pos.append(p)

    # bucket atoms by forced-W1 count f1 (slots with pos < W2B); per-f1
    # group padded to tiles of 128, group tile-counts equalized across cores.
    # Slot split per atom: the f1 forced-W1 slots gather via W1; the rest
    # (free or forced-W2) via W2 (valid: free slots have pos >= W2B).
    atom_f1 = []
    for k in range(N_CORES):
        f1 = np.zeros(6401, np.int64)
        slot_w1 = np.zeros((6401, 6), bool)
        for a in range(6401):
            for c in range(6):
                b = agg_draws[k][a, c]
                pp = pos[k][b] if b >= 0 else (WIN + 1)
                if pp < W2B:
                    slot_w1[a, c] = True
            f1[a] = slot_w1[a].sum()
        atom_f1.append((f1, slot_w1))
    grp_tiles = [max(-(-int((atom_f1[k][0] == f).sum()) // 128) or 0
                     for k in range(N_CORES)) for f in range(7)]
    # ensure nonzero handled: groups with zero atoms on all cores get 0 tiles
    for f in range(7):
        if all((atom_f1[k][0] == f).sum() == 0 for k in range(N_CORES)):
            grp_tiles[f] = 0
    AT2 = sum(grp_tiles)
    ktile = []
    for f in range(7):
        ktile += [f] * grp_tiles[f]
    # per-core permutation: new row -> old local atom (or -1 pad)
    perm_new2old = []
    perm_old2new = []
    for k in range(N_CORES):
        f1 = atom_f1[k][0]
        order = np.full(AT2 * 128, -1, np.int64)
        o2n = np.zeros(6401, np.int64)
        base = 0
        for f in range(7):
            rows = np.nonzero(f1 == f)[0]
            order[base:base + len(rows)] = rows
            o2n[rows] = base + np.arange(len(rows))
            base += grp_tiles[f] * 128
        perm_new2old.append(order)
        perm_old2new.append(o2n)

    # order bonds within each core: window-1 rev draws first, then window-2,
    # each group padded to a tile boundary. Recompute BT for group padding.
    bond_wflag = []
    new_bonds = []
    for k in range(N_CORES):
        bk = bonds[k]
        rd = b2revb[bk]
        p = np.array([pos[k][b] if b >= 0 else 0 for b in rd])
        isw2 = p >= WIN
        g1b, g2b = bk[~isw2], bk[isw2]
        n1t = -(-len(g1b) // 128)
        n2t = -(-len(g2b) // 128)
        new_bonds.append((g1b, g2b, n1t, n2t))
    NT1 = max(n1t for (_, _, n1t, n2t) in new_bonds)
    NT2 = max(n2t for (_, _, n1t, n2t) in new_bonds)
    BT = NT1 + NT2
    NBND = BT * 128
    bonds2 = []
    for k in range(N_CORES):
        g1b, g2b, _, _ = new_bonds[k]
        b_ord = np.full(NBND, -1, np.int64)
        b_ord[:len(g1b)] = g1b
        b_ord[NT1 * 128:NT1 * 128 + len(g2b)] = g2b
        wf = np.zeros(BT, np.int64)
        wf[NT1:] = 1
        bond_wflag.append(wf)
        bonds2.append(b_ord)
    # rebuild rev_draws in the new order
    rev_draws = []
    for k in range(N_CORES):
        d2 = np.full(NBND, -1, np.int64)
        valid = bonds2[k] >= 0
        d2[valid] = b2revb[bonds2[k][valid]]
        rev_draws.append(d2)
    bonds = [b[b >= 0] for b in bonds2]
    bonds_full = bonds2

    # local row of bond b on its owner core (position in new order)
    lrow = np.zeros(NB, np.int64)
    for k in range(N_CORES):
        v = bonds_full[k] >= 0
        lrow[bonds_full[k][v]] = np.nonzero(v)[0]

    out = []
    for k in range(N_CORES):
        d = {}
        # send list: for each dest kk, local rows of I[k][kk] padded to S_pad
        snd = np.zeros((N_CORES, S_pad), np.int64)
        for kk in range(N_CORES):
            ii = I[k][kk]
            snd[kk, :len(ii)] = lrow[ii]
        d["snd"] = _wrap_idx(snd.ravel())
        d["n_snd"] = N_CORES * S_pad

        def windows(draws_flat, pk):
            """draws (global bond ids, -1=dummy) -> (w1 idx, w2 idx) lists."""
            n = len(draws_flat)
            w1 = np.zeros(n, np.int64)
            w2 = np.full(n, 32767, np.int64)  # z1 relative pos = NT-1-W2B
            z1rel = NT - 1 - W2B
            w2[:] = z1rel
            for i, b in enumerate(draws_flat):
                if b < 0:
                    continue
                p = pk[b]
                if p < WIN:
                    w1[i] = p
                else:
                    w2[i] = p - W2B
            return w1, w2

        # atom agg, bucketed: tile with k=f: W1 part = k cols, W2 = 6-k cols
        f1, slot_w1 = atom_f1[k]
        aW1, aW2 = [], []
        z1rel = NT - 1 - W2B
        for ti in range(AT2):
            kk_ = ktile[ti]
            w1c = np.zeros((kk_, 128), np.int64)
            w2c = np.full((6 - kk_, 128), z1rel, np.int64)
            for p128 in range(128):
                old = perm_new2old[k][ti * 128 + p128]
                if old < 0:
                    w1c[:, p128] = 0
                    continue
                c1 = c2 = 0
                for c in range(6):
                    b = agg_draws[k][old, c]
                    pp = pos[k][b] if b >= 0 else None
                    if b >= 0 and pp < W2B:
                        w1c[c1, p128] = pp; c1 += 1
                    elif b >= 0:
                        w2c[c2, p128] = (pp - W2B); c2 += 1
                    else:
                        w2c[c2, p128] = z1rel; c2 += 1
            aW1.append(w1c.ravel()); aW2.append(w2c.ravel())
        d["aggW1"] = _wrap_idx(np.concatenate(aW1) if aW1 else np.zeros(16))
        d["aggW2"] = _wrap_idx(np.concatenate(aW2) if aW2 else np.zeros(16))
        # rev: bonds were ordered so each tile is window-pure; emit single
        # per-tile idx lists plus the per-tile window flag
        w1, w2 = windows(rev_draws[k], pos[k])
        wflag = bond_wflag[k]
        rev_single = np.where(np.repeat(wflag, 128) == 0, w1, w2)
        d["rev"] = _wrap_idx(rev_single)
        # g1: local atom row of b2a (transpose-mode, own-ma table)
        g1 = np.zeros(NBND, np.int64)
        v = bonds_full[k] >= 0
        bv = bonds_full[k][v]
        g1[np.nonzero(v)[0]] = perm_old2new[k][np.where(b2a[bv] >= 1, (b2a[bv] - 1) % 6400, 6400)]
        d["g1"] = _wrap_idx(g1)
        # F2 ma/ia rows: tile t -> local rows m*A+t (transpose-mode gathers)
        f2r = np.zeros((A, NMOL), np.int64)
        for t in range(A):
            f2r[t] = perm_old2new[k][np.arange(NMOL) * A + t]
        d["f2r"] = _wrap_idx(f2r.ravel())
        # F2 agg: atoms in (mol, t) tiles: tile t rows = atom local m*A+t
        fd = np.full((A, 128, 6), -1, np.int64)
        for t in range(A):
            rows = np.arange(NMOL) * A + t
            fd[t] = a2b[6400 * k + 1 + rows]
        fcol = fd.transpose(0, 2, 1).reshape(-1)
        w1, w2 = windows(fcol, pos[k])
        d["fW1"], d["fW2"] = _wrap_idx(w1), _wrap_idx(w2)
        d["bonds_full"] = bonds_full[k]
        d["perm"] = perm_new2old[k]
        out.append(d)
    return out, S_pad, BT, NT, W2B, NT1, AT2, ktile


# ======================= weights =========================================

def prep_weights(inp, BT):
    w = {kk: np.asarray(inp[kk], np.float32) for kk in
         ["W_i_atom", "W_i_bond", "W_h", "W_lr", "gru_bias", "w_ih_f", "w_hh_f",
          "b_ih_f", "b_hh_f", "w_ih_b", "w_hh_b", "b_ih_b", "b_hh_b", "W_o", "b_o"]}
    bf = lambda x: np.ascontiguousarray(x).astype(ml_dtypes.bfloat16)
    out = {}
    Wia = np.zeros((256, HP), np.float32); Wia[:AFD, :H] = w["W_i_atom"]
    Wib = np.zeros((256, HP), np.float32); Wib[:BFD, :H] = w["W_i_bond"]
    out["Wia"], out["Wib"] = bf(Wia), bf(Wib)
    for dd in range(DEPTH - 1):
        Wh = np.zeros((HP, HP), np.float32); Wh[:H, :H] = w["W_h"][dd]
        out[f"Wh{dd}"] = bf(Wh)
    for p in range(3):
        out[f"Wlr{p}"] = bf(w["W_lr"][p * H:(p + 1) * H])
    out["grub"] = bf(w["gru_bias"][None, :])
    for dd in "fb":
        out[f"wihT{dd}"] = bf(w[f"w_ih_{dd}"].T)
        out[f"whhT{dd}"] = bf(w[f"w_hh_{dd}"].T)
        out[f"bih{dd}"] = bf(w[f"b_ih_{dd}"][None, :])
        out[f"bhh{dd}"] = bf(w[f"b_hh_{dd}"][None, :])
    Wo_pad = np.zeros((6 * 128, H), np.float32)
    for i, (k0, k1) in enumerate(KCH):
        Wo_pad[i * 128:i * 128 + (k1 - k0)] = w["W_o"][k0:k1]
        Wo_pad[(3 + i) * 128:(3 + i) * 128 + (k1 - k0)] = w["W_o"][H + k0:H + k1]
    out["Wo"] = bf(Wo_pad)
    out["boT"] = np.ascontiguousarray(w["b_o"][:, None]).astype(np.float32)
    return out


WSPEC = (
    [("Wia", (256, HP)), ("Wib", (256, HP))]
    + [(f"Wh{d}", (HP, HP)) for d in range(DEPTH - 1)]
    + [(f"Wlr{p}", (H, H)) for p in range(3)]
    + [("grub", (1, H))]
    + [(f"wihT{d}", (H, 3 * H)) for d in "fb"]
    + [(f"whhT{d}", (H, 3 * H)) for d in "fb"]
    + [(f"bih{d}", (1, 3 * H)) for d in "fb"]
    + [(f"bhh{d}", (1, 3 * H)) for d in "fb"]
    + [("Wo", (6 * 128, H))]
)


# ======================= kernel build ====================================

def build_kernel(S_pad, BT, NT, W2B, NT1, AT2, ktile, mp_iters=DEPTH - 1, do_final=True):
    NBND = BT * 128
    nc = bacc.Bacc("TRN2", target_bir_lowering=False, debug=False,
                   num_devices=N_CORES)

    faT = nc.dram_tensor("faT", [256, AT2 * 128], BF16, kind="ExternalInput")
    fbT = nc.dram_tensor("fbT", [256, NBND], BF16, kind="ExternalInput")
    idx_specs = {
        "snd": 8 * S_pad,
        "aggW1": sum(ktile) * 128, "aggW2": (6 * AT2 - sum(ktile)) * 128,
        "rev": NBND, "g1": NBND,
        "fW1": A * 768, "fW2": A * 768, "f2r": A * NMOL,
    }
    idr = {nm: nc.dram_tensor(nm, [128, -(-n // 16)], I16, kind="ExternalInput")
           for nm, n in idx_specs.items()}
    wdr = {nm: nc.dram_tensor(nm, list(sh), BF16 if nm != "boT" else F32,
                              kind="ExternalInput")
           for nm, sh in WSPEC}
    wdr["boT"] = nc.dram_tensor("boT", [H, 1], F32, kind="ExternalInput")
    mv_t = nc.dram_tensor("mv_t", [H, NMOL], F32, kind="ExternalOutput")

    # internal DRAM
    TU = nc.dram_tensor("t_union", [NT, HP], BF16)
    SND = nc.dram_tensor("sendb", [8 * S_pad, HP], BF16)
    MB = nc.dram_tensor("own_mb", [NBND, HP], BF16)
    MAb = nc.dram_tensor("own_ma", [AT2 * 128, HP], BF16)
    MAf = nc.dram_tensor("ma_f32", [AT2 * 128, H], F32)
    IA = nc.dram_tensor("ia_t", [AT2 * 128, HP], BF16)
    IB = nc.dram_tensor("ib_t", [NBND, HP], BF16)
    msg_d = nc.dram_tensor("msg_d", [A, NMOL, H], BF16)
    gi_d = {d: nc.dram_tensor(f"gi_{d}", [A, NMOL, 3 * H], BF16) for d in "fb"}
    hT_d = {d: nc.dram_tensor(f"hT_{d}", [A, 3, 128, NMOL], BF16) for d in "fb"}

    rg = [list(range(N_CORES))]

    with tile.TileContext(nc) as tc:
        with tc.tile_pool(name="const", bufs=1) as cp:
            nc.gpsimd.load_library(mlp)
            ident = cp.tile([128, 128], F32, tag="ident")
            make_identity(nc, ident[:])
            identb = cp.tile([128, 128], BF16, tag="identb")
            nc.vector.tensor_copy(out=identb[:], in_=ident[:])
            ones = cp.tile([1, 128], BF16, tag="ones")
            nc.vector.memset(ones[:], 1.0)
            zrow = cp.tile([2, HP], BF16, tag="zrow")
            nc.vector.memset(zrow[:], 0.0)
            nc.sync.dma_start(out=TU[0:1, :], in_=zrow[:1, :])
            nc.sync.dma_start(out=TU[NT - 1:NT, :], in_=zrow[1:2, :])

            idx = {}
            for nm in idx_specs:
                t = cp.tile(list(idr[nm].shape), I16, tag=f"ix_{nm}")
                nc.sync.dma_start(out=t[:], in_=idr[nm][:])
                idx[nm] = t

            def wchunks(nm, dt=BF16):
                dr = wdr[nm]
                outs = []
                for i, k0 in enumerate(range(0, dr.shape[0], 128)):
                    k1 = min(k0 + 128, dr.shape[0])
                    t = cp.tile([k1 - k0, dr.shape[1]], dt, tag=f"{nm}_{i}")
                    nc.sync.dma_start(out=t[:], in_=dr[k0:k1, :])
                    outs.append(t)
                return outs

            def load1(nm, dt=BF16):
                t = cp.tile(list(wdr[nm].shape), dt, tag=nm)
                nc.sync.dma_start(out=t[:], in_=wdr[nm][:])
                return t

            Wia = wchunks("Wia")
            Wib = wchunks("Wib")
            Wh = [wchunks(f"Wh{d}") for d in range(DEPTH - 1)]
            Wlr = [wchunks(f"Wlr{p}") for p in range(3)]
            grub = load1("grub")
            wihT = {d: wchunks(f"wihT{d}") for d in "fb"}
            whhT = {d: wchunks(f"whhT{d}") for d in "fb"}
            bih = {d: load1(f"bih{d}") for d in "fb"}
            bhh = {d: load1(f"bhh{d}") for d in "fb"}
            WoC = wchunks("Wo")
            boTc = wchunks("boT", dt=F32)

            # ---------------- exchange helper -----------------------------
            def send_and_a2a(pool):
                for blk in range(8):
                    g = pool.tile([128, S_pad // 128, HP], BF16, tag="sndg")
                    for p0 in range(0, S_pad, 1024):
                        n = min(1024, S_pad - p0)
                        nc.gpsimd.dma_gather(
                            g[:, p0 // 128:(p0 + n) // 128, :], MB[:, :],
                            idx["snd"][:, (blk * S_pad + p0) // 16:
                                       (blk * S_pad + p0 + n) // 16],
                            n, n, HP)
                    nc.sync.dma_start(
                        out=SND.ap()[blk * S_pad:(blk + 1) * S_pad, :]
                            .rearrange("(a p) h -> p a h", p=128),
                        in_=g[:])
                nc.gpsimd.collective_compute(
                    "AllToAll", ALU.bypass, replica_groups=rg,
                    ins=[SND[:, :]], outs=[TU[1:1 + 8 * S_pad, :]])

            # ---------------- stage 0: input projections -----------------
            def proj(pool, psum, src_t, Wch, t0, bt, outs_bf, out_f32):
                """project bt tiles starting at t0; outs_bf: list of (dram, off)"""
                lhs = pool.tile([128, 2, bt * 128], BF16, tag="plhs")
                nc.sync.dma_start(
                    out=lhs[:],
                    in_=src_t.ap()[:, t0 * 128:(t0 + bt) * 128]
                        .rearrange("(c p) n -> p c n", p=128))
                stg = pool.tile([128, bt, HP], BF16, tag="pstg")
                stgf = None
                if out_f32:
                    stgf = pool.tile([128, bt, H], F32, tag="pstgf")
                for i in range(bt):
                    pm = psum.tile([128, HP], F32, tag="pmm")
                    for c in range(2):
                        nc.tensor.matmul(out=pm[:], lhsT=lhs[:, c, i * 128:(i + 1) * 128],
                                         rhs=Wch[c][:], start=(c == 0), stop=(c == 1),
                                         skip_group_check=True)
                    nc.scalar.activation(out=stg[:, i, :], in_=pm[:], func=AF.Relu)
                    if out_f32:
                        nc.scalar.activation(out=stgf[:, i, :],
                                             in_=pm[:, :H], func=AF.Relu)
                for dr, off in outs_bf:
                    nc.sync.dma_start(
                        out=dr.ap()[(t0) * 128:(t0 + bt) * 128, :]
                            .rearrange("(a p) h -> p a h", p=128),
                        in_=stg[:])
                if out_f32:
                    nc.sync.dma_start(
                        out=MAf.ap()[t0 * 128:(t0 + bt) * 128, :]
                            .rearrange("(a p) h -> p a h", p=128),
                        in_=stgf[:])

            PB = 4
            with (tc.tile_pool(name="s0", bufs=3) as pool,
                  tc.tile_pool(name="s0p", bufs=4, space="PSUM") as psum,
                  tc.tile_pool(name="snd0e", bufs=2) as sndp):
                for t0 in range(0, BT, PB):
                    bt = min(PB, BT - t0)
                    proj(pool, psum, fbT, Wib, t0, bt, [(MB, 0), (IB, 0)], False)
                send_and_a2a(sndp)
                for t0 in range(0, AT2, PB):
                    bt = min(PB, AT2 - t0)
                    proj(pool, psum, faT, Wia, t0, bt, [(MAb, 0), (IA, 0)], True)

            w1rows = min(NT, WIN)
            w2rows = NT - W2B

            # ---------------- message passing rounds ---------------------
            AB = 3    # atom tiles per batch
            BB = 4    # bond tiles per batch
            kcum1 = [0]
            kcum2 = [0]
            for f in ktile:
                kcum1.append(kcum1[-1] + f * 8)       # idx cols (128 idx = 8)
                kcum2.append(kcum2[-1] + (6 - f) * 8)
            for r in range(mp_iters):
                with (tc.tile_pool(name=f"A{r}", bufs=2) as pool,):
                    for t0 in range(0, AT2, AB):
                        bt = min(AB, AT2 - t0)
                        g1 = pool.tile([128, bt * 6, HP], BF16, tag="G1")
                        g2 = pool.tile([128, bt * 6, HP], BF16, tag="G2")
                        for i in range(bt):
                            kk_ = ktile[t0 + i]
                            if kk_ > 0:
                                nc.gpsimd.dma_gather(
                                    g1[:, i * 6:i * 6 + kk_, :], TU[0:w1rows, :],
                                    idx["aggW1"][:, kcum1[t0 + i]:kcum1[t0 + i + 1]],
                                    kk_ * 128, kk_ * 128, HP)
                            if kk_ < 6:
                                nc.gpsimd.dma_gather(
                                    g2[:, i * 6:i * 6 + 6 - kk_, :], TU[W2B:NT, :],
                                    idx["aggW2"][:, kcum2[t0 + i]:kcum2[t0 + i + 1]],
                                    (6 - kk_) * 128, (6 - kk_) * 128, HP)
                        maf = pool.tile([128, bt, H], F32, tag="maf")
                        nc.sync.dma_start(
                            in_=MAf.ap()[t0 * 128:(t0 + bt) * 128, :]
                                .rearrange("(a p) h -> p a h", p=128),
                            out=maf[:])
                        mab = pool.tile([128, bt, HP], BF16, tag="mab")
                        nc.vector.memset(mab[:], 0.0)
                        for i in range(bt):
                            kk_ = ktile[t0 + i]
                            ops = [g1[:, i * 6 + c, :H] for c in range(kk_)] +                                   [g2[:, i * 6 + c, :H] for c in range(6 - kk_)]
                            ssum = pool.tile([128, H], F32, tag="ssum")
                            nc.vector.tensor_add(out=ssum[:], in0=ops[0], in1=ops[1])
                            smax = pool.tile([128, H], F32, tag="smax")
                            nc.vector.tensor_tensor(out=smax[:], in0=ops[0],
                                                    in1=ops[1], op=ALU.max)
                            for c in range(2, 6):
                                nc.vector.tensor_add(out=ssum[:], in0=ssum[:],
                                                     in1=ops[c])
                                nc.vector.tensor_tensor(out=smax[:], in0=smax[:],
                                                        in1=ops[c], op=ALU.max)
                            nc.vector.tensor_mul(out=ssum[:], in0=ssum[:], in1=smax[:])
                            nc.vector.tensor_add(out=maf[:, i, :],
                                                 in0=maf[:, i, :],
                                                 in1=ssum[:])
                            nc.vector.tensor_copy(out=mab[:, i, :H],
                                                  in_=maf[:, i, :])
                        nc.sync.dma_start(
                            out=MAf.ap()[t0 * 128:(t0 + bt) * 128, :]
                                .rearrange("(a p) h -> p a h", p=128),
                            in_=maf[:])
                        nc.sync.dma_start(
                            out=MAb.ap()[t0 * 128:(t0 + bt) * 128, :]
                                .rearrange("(a p) h -> p a h", p=128),
                            in_=mab[:])

                with (tc.tile_pool(name=f"B{r}", bufs=3) as pool,
                      tc.tile_pool(name=f"B{r}p", bufs=4, space="PSUM") as psum):
                    bstarts = []
                    for t0 in range(0, NT1, BB):
                        bstarts.append((t0, min(BB, NT1 - t0)))
                    for t0 in range(NT1, BT, BB):
                        bstarts.append((t0, min(BB, BT - t0)))
                    for t0, bt in bstarts:
                        n = bt * 128
                        g1t = pool.tile([128, 3, n], BF16, tag="g1t")
                        nc.gpsimd.dma_gather(
                            g1t[:], MAb[:, :],
                            idx["g1"][:, t0 * 8:t0 * 8 + n // 16], n, n, HP,
                            transpose=True)
                        g2a = pool.tile([128, 3, n], BF16, tag="g2a")
                        src = TU[0:w1rows, :] if t0 < NT1 else TU[W2B:NT, :]
                        nc.gpsimd.dma_gather(
                            g2a[:], src,
                            idx["rev"][:, t0 * 8:t0 * 8 + n // 16], n, n, HP,
                            transpose=True)
                        nc.vector.tensor_sub(out=g1t[:], in0=g1t[:], in1=g2a[:])
                        ibt = pool.tile([128, bt, HP], BF16, tag="ibt")
                        nc.sync.dma_start(
                            out=ibt[:],
                            in_=IB.ap()[t0 * 128:(t0 + bt) * 128, :]
                                .rearrange("(a p) h -> p a h", p=128))
                        stg = pool.tile([128, bt, HP], BF16, tag="mstg")
                        for i in range(bt):
                            pm = psum.tile([128, HP], F32, tag="bmm")
                            for c in range(3):
                                nc.tensor.matmul(
                                    out=pm[:], lhsT=g1t[:, c, i * 128:(i + 1) * 128],
                                    rhs=Wh[r][c][:], start=(c == 0), stop=(c == 2),
                                    skip_group_check=True)
                            pre = pool.tile([128, HP], F32, tag="pre")
                            nc.vector.tensor_add(out=pre[:], in0=pm[:], in1=ibt[:, i, :])
                            nc.scalar.activation(out=stg[:, i, :], in_=pre[:],
                                                 func=AF.Relu)
                        nc.sync.dma_start(
                            out=MB.ap()[t0 * 128:(t0 + bt) * 128, :]
                                .rearrange("(a p) h -> p a h", p=128),
                            in_=stg[:])

                with tc.tile_pool(name=f"X{r}", bufs=2) as pool:
                    send_and_a2a(pool)

            if not do_final:
                nc.compile()
                return nc

            # ---------------- final readout + GRU + output ----------------
            def transpose_chunks(pool, psum, src, dtype=BF16, eng="act", tag="tx"):
                outs = []
                for ci, (k0, k1) in enumerate(KCH):
                    pt = psum.tile([128, 128], src.dtype, tag="tp")
                    idt = ident if src.dtype == F32 else identb
                    nc.tensor.transpose(out=pt[:k1 - k0, :], in_=src[:, k0:k1],
                                        identity=idt[:])
                    st = pool.tile([128, 128], dtype, tag=f"{tag}{ci}")
                    if eng == "act":
                        nc.scalar.copy(out=st[:k1 - k0, :], in_=pt[:k1 - k0, :])
                    else:
                        nc.vector.tensor_copy(out=st[:k1 - k0, :], in_=pt[:k1 - k0, :])
                    outs.append(st)
                return outs

            def mm_kchunks(pt, lhsT_tiles, rhs_tiles, start=True, stop=True,
                           n0=0, n1=H):
                for ci, (lt, rt) in enumerate(zip(lhsT_tiles, rhs_tiles)):
                    klen = min(lt.shape[0], rt.shape[0])
                    nc.tensor.matmul(out=pt[:], lhsT=lt[:klen, :],
                                     rhs=rt[:klen, n0:n1],
                                     start=(start and ci == 0),
                                     stop=(stop and ci == len(lhsT_tiles) - 1),
                                     skip_group_check=True)

            ma_r = MAf.ap()[0:ATOMS, :].rearrange("(m t) h -> m t h", t=A)
            ia_r = IA.ap()[0:ATOMS, :].rearrange("(m t) h -> m t h", t=A)
            with tc.tile_pool(name="h0p", bufs=1) as h0p:
                h0 = h0p.tile([128, H], F32, tag="h0")
                nc.vector.memset(h0[:], -1e30)
                FB = 3
                with (tc.tile_pool(name="F", bufs=2) as pool,
                      tc.tile_pool(name="Fp", bufs=2, space="PSUM") as psum,
                      tc.tile_pool(name="Fg", bufs=2, space="PSUM") as psg):
                    for t0 in range(0, A, FB):
                        bt = min(FB, A - t0)
                        g1 = pool.tile([128, bt * 6, HP], BF16, tag="fG1")
                        g2 = pool.tile([128, bt * 6, HP], BF16, tag="fG2")
                        for i in range(bt):
                            c0 = (t0 + i) * 48
                            nc.gpsimd.dma_gather(
                                g1[:, i * 6:(i + 1) * 6, :], TU[0:w1rows, :],
                                idx["fW1"][:, c0:c0 + 48], 768, 768, HP)
                            nc.gpsimd.dma_gather(
                                g2[:, i * 6:(i + 1) * 6, :], TU[W2B:NT, :],
                                idx["fW2"][:, c0:c0 + 48], 768, 768, HP)
                        for i in range(bt):
                            t = t0 + i
                            G1 = g1[:, i * 6:(i + 1) * 6, :]
                            G2 = g2[:, i * 6:(i + 1) * 6, :]
                            s3 = pool.tile([128, 3, H], BF16, tag="fs3")
                            nc.vector.tensor_add(out=s3[:], in0=G1[:, 0:3, :H],
                                                 in1=G1[:, 3:6, :H])
                            t3 = pool.tile([128, 3, H], BF16, tag="ft3")
                            nc.vector.tensor_add(out=t3[:], in0=G2[:, 0:3, :H],
                                                 in1=G2[:, 3:6, :H])
                            nc.vector.tensor_add(out=s3[:], in0=s3[:], in1=t3[:])
                            ssum = pool.tile([128, H], F32, tag="fsum")
                            nc.vector.tensor_add(out=ssum[:], in0=s3[:, 0, :],
                                                 in1=s3[:, 1, :])
                            nc.vector.tensor_add(out=ssum[:], in0=ssum[:],
                                                 in1=s3[:, 2, :])
                            m3 = pool.tile([128, 3, H], BF16, tag="fm3")
                            nc.vector.tensor_tensor(out=m3[:], in0=G1[:, 0:3, :H],
                                                    in1=G1[:, 3:6, :H], op=ALU.max)
                            m32 = pool.tile([128, 3, H], BF16, tag="fm32")
                            nc.vector.tensor_tensor(out=m32[:], in0=G2[:, 0:3, :H],
                                                    in1=G2[:, 3:6, :H], op=ALU.max)
                            nc.vector.tensor_tensor(out=m3[:], in0=m3[:], in1=m32[:],
                                                    op=ALU.max)
                            smax = pool.tile([128, H], F32, tag="fmax")
                            nc.vector.tensor_tensor(out=smax[:], in0=m3[:, 0, :],
                                                    in1=m3[:, 1, :], op=ALU.max)
                            nc.vector.tensor_tensor(out=smax[:], in0=smax[:],
                                                    in1=m3[:, 2, :], op=ALU.max)
                            agg = pool.tile([128, H], BF16, tag="fagg")
                            nc.vector.tensor_mul(out=agg[:], in0=ssum[:], in1=smax[:])
                            lhs_a = transpose_chunks(pool, psum, agg, tag="ta")
                            maT = pool.tile([128, 3, NMOL], BF16, tag="fmaT")
                            nc.gpsimd.dma_gather(
                                maT[:], MAb[:, :],
                                idx["f2r"][:, t * 8:(t + 1) * 8], NMOL, NMOL, HP,
                                transpose=True)
                            iaT = pool.tile([128, 3, NMOL], BF16, tag="fiaT")
                            nc.gpsimd.dma_gather(
                                iaT[:], IA[:, :],
                                idx["f2r"][:, t * 8:(t + 1) * 8], NMOL, NMOL, HP,
                                transpose=True)
                            pm = psum.tile([128, H], F32, tag="fmm")
                            mm_kchunks(pm, lhs_a, Wlr[0], start=True, stop=False)
                            for ci, (k0, k1) in enumerate(KCH):
                                nc.tensor.matmul(
                                    out=pm[:], lhsT=maT[:k1 - k0, ci, :],
                                    rhs=Wlr[1][ci][:k1 - k0, :],
                                    start=False, stop=False,
                                    skip_group_check=True)
                            for ci, (k0, k1) in enumerate(KCH):
                                nc.tensor.matmul(
                                    out=pm[:], lhsT=iaT[:k1 - k0, ci, :],
                                    rhs=Wlr[2][ci][:k1 - k0, :],
                                    start=False, stop=(ci == 2),
                                    skip_group_check=True)
                            hid = pool.tile([128, H], F32, tag="fhid")
                            nc.vector.tensor_copy(out=hid[:], in_=pm[:])
                            nc.vector.tensor_tensor(out=h0[:], in0=h0[:], in1=hid[:],
                                                    op=ALU.max)
                            nc.tensor.matmul(out=pm[:], lhsT=ones[:, :], rhs=grub[:, :],
                                             start=False, stop=True,
                                             skip_group_check=True)
                            msg = pool.tile([128, H], BF16, tag="fmsg")
                            nc.scalar.activation(out=msg[:], in_=pm[:], func=AF.Relu)
                            nc.sync.dma_start(out=msg_d[t, :, :], in_=msg[:])
                            lhs_x = transpose_chunks(pool, psum, msg, tag="txx")
                            for di in "fb":
                                gts = pool.tile([128, 3 * H], BF16, tag="fgts")
                                for g in range(3):
                                    pg = psg.tile([128, H], F32, tag="fgi")
                                    mm_kchunks(pg, lhs_x, wihT[di], start=True,
                                               stop=False, n0=g * H, n1=(g + 1) * H)
                                    nc.tensor.matmul(
                                        out=pg[:], lhsT=ones[:, :],
                                        rhs=bih[di][:, g * H:(g + 1) * H],
                                        start=False, stop=(g == 2),
                                        skip_group_check=True)
                                    if g < 2:
                                        nc.tensor.matmul(
                                            out=pg[:], lhsT=ones[:, :],
                                            rhs=bhh[di][:, g * H:(g + 1) * H],
                                            start=False, stop=True,
                                            skip_group_check=True)
                                    nc.scalar.copy(out=gts[:, g * H:(g + 1) * H],
                                                   in_=pg[:])
                                nc.sync.dma_start(out=gi_d[di][t, :, :], in_=gts[:])

                # ---- GRU ---------------------------------------------------
                with (tc.tile_pool(name="gruh", bufs=2) as hp,
                      tc.tile_pool(name="gru", bufs=3) as gp,
                      tc.tile_pool(name="grut", bufs=2) as gtp,
                      tc.tile_pool(name="grutp", bufs=2, space="PSUM") as ptp,
                      tc.tile_pool(name="grup", bufs=2, space="PSUM") as pgh):
                    def hT_one(hsrc, tag):
                        big = gtp.tile([128, 3, 128], BF16, tag=tag)
                        nc.vector.memset(big[:, 2, :], 0.0)
                        for ci, (k0, k1) in enumerate(KCH):
                            pt = ptp.tile([128, 128], F32, tag="gtp")
                            nc.tensor.transpose(out=pt[:k1 - k0, :],
                                                in_=hsrc[:, k0:k1],
                                                identity=ident[:])
                            nc.vector.tensor_copy(out=big[:k1 - k0, ci, :],
                                                  in_=pt[:k1 - k0, :])
                        return big
                    hcur, hT = {}, {}
                    for di in "fb":
                        ht = hp.tile([128, H], F32, tag=f"h_{di}")
                        nc.vector.tensor_copy(out=ht[:], in_=h0[:])
                        hcur[di] = ht
                        hT[di] = hT_one(ht, f"hx{di}")
                    for step in range(A):
                        for di in "fb":
                            t = step if di == "f" else A - 1 - step
                            h = hcur[di]
                            gh = []
                            for g in range(3):
                                pg = pgh.tile([128, H], F32, tag=f"gh{g}")
                                for ci, (k0, k1) in enumerate(KCH):
                                    nc.tensor.matmul(
                                        out=pg[:], lhsT=hT[di][:k1 - k0, ci, :],
                                        rhs=whhT[di][ci][:k1 - k0, g * H:(g + 1) * H],
                                        start=(ci == 0), stop=False,
                                        skip_group_check=True)
                                nc.tensor.matmul(
                                    out=pg[:], lhsT=ones[:, :],
                                    rhs=bhh[di][:, g * H:(g + 1) * H],
                                    start=False, stop=True,
                                    skip_group_check=True)
                                gh.append(pg)
                            giw = gp.tile([128, 3 * H], BF16, tag="gil")
                            nc.sync.dma_start(out=giw[:], in_=gi_d[di][t, :, :])
                            gi = [giw[:, g * H:(g + 1) * H] for g in range(3)]
                            rz = gp.tile([128, 2 * H], F32, tag="rz")
                            nc.vector.tensor_add(out=rz[:, 0:H], in0=gi[0], in1=gh[0][:])
                            nc.vector.tensor_add(out=rz[:, H:2 * H], in0=gi[1],
                                                 in1=gh[1][:])
                            nc.scalar.activation(out=rz[:], in_=rz[:], func=AF.Sigmoid)
                            r_ = rz[:, 0:H]
                            z = rz[:, H:2 * H]
                            n_ = gp.tile([128, H], F32, tag="n")
                            nc.vector.tensor_mul(out=n_[:], in0=r_, in1=gh[2][:])
                            nc.vector.tensor_add(out=n_[:], in0=n_[:], in1=gi[2])
                            nc.scalar.activation(out=n_[:], in_=n_[:], func=AF.Tanh)
                            hn = hp.tile([128, H], F32, tag=f"h_{di}")
                            nc.vector.tensor_sub(out=hn[:], in0=h[:], in1=n_[:])
                            nc.vector.tensor_mul(out=hn[:], in0=z, in1=hn[:])
                            nc.vector.tensor_add(out=hn[:], in0=hn[:], in1=n_[:])
                            hcur[di] = hn
                            hT[di] = hT_one(hn, f"hx{di}")
                            nc.sync.dma_start(
                                out=hT_d[di].ap()[t, :, :, :]
                                    .rearrange("c p n -> p c n"),
                                in_=hT[di][:])

                # ---- F4: output projection + mol mean ----------------------
                with (tc.tile_pool(name="F4", bufs=3) as pool4,
                      tc.tile_pool(name="F4a", bufs=1) as accp,
                      tc.tile_pool(name="F4p", bufs=2, space="PSUM") as psum4):
                    mvacc = []
                    for ci, (k0, k1) in enumerate(KCH):
                        mt = accp.tile([128, NMOL], F32, tag=f"mv{ci}")
                        nc.vector.memset(mt[:], 0.0)
                        mvacc.append(mt)
                    for t in sorted(range(A), key=lambda q: max(q, A - 1 - q)):
                        cat = []
                        for di in "fb":
                            ct = pool4.tile([128, 3, NMOL], BF16, tag=f"c{di}")
                            nc.sync.dma_start(
                                out=ct[:],
                                in_=hT_d[di].ap()[t, :, :, :]
                                    .rearrange("c p n -> p c n"))
                            for ci, (k0, k1) in enumerate(KCH):
                                cat.append((ct[:k1 - k0, ci, :], k1 - k0))
                        for mc, (m0, m1) in enumerate(KCH):
                            pm = psum4.tile([128, NMOL], F32, tag="f4mm")
                            for ii, (ctv, klen) in enumerate(cat):
                                nc.tensor.matmul(
                                    out=pm[:m1 - m0, :],
                                    lhsT=WoC[ii][:klen, m0:m1],
                                    rhs=ctv[:],
                                    start=(ii == 0), stop=(ii == 5),
                                    skip_group_check=True)
                            ah = pool4.tile([128, NMOL], F32, tag="ah")
                            nc.scalar.activation(out=ah[:m1 - m0, :],
                                                 in_=pm[:m1 - m0, :], func=AF.Relu,
                                                 bias=boTc[mc][:m1 - m0, :])
                            nc.vector.tensor_add(out=mvacc[mc][:m1 - m0, :],
                                                 in0=mvacc[mc][:m1 - m0, :],
                                                 in1=ah[:m1 - m0, :])
                    for mc, (m0, m1) in enumerate(KCH):
                        nc.vector.tensor_scalar_mul(mvacc[mc][:m1 - m0, :],
                                                    mvacc[mc][:m1 - m0, :], 1.0 / A)
                        nc.sync.dma_start(out=mv_t[m0:m1, :],
                                          in_=mvacc[mc][:m1 - m0, :])

    nc.compile()
    return nc


# ======================= host entry ======================================

_CACHE = {}


def prep_inputs(inp, rt, BT):
    f_atoms = np.asarray(inp["f_atoms"], np.float32)
    f_bonds = np.asarray(inp["f_bonds"], np.float32)
    NBND = BT * 128
    ins = []
    for k in range(N_CORES):
        d = dict(rt[k])
        bonds_full = d.pop("bonds_full")
        perm = d.pop("perm")
        AT2 = len(perm) // 128
        old_ids = np.concatenate([np.arange(6400 * k + 1, 6400 * (k + 1) + 1),
                                  [0]])
        fa = np.zeros((AT2 * 128, 256), np.float32)
        vv = perm >= 0
        fa[np.nonzero(vv)[0], :AFD] = f_atoms[old_ids[perm[vv]]]
        fb = np.zeros((NBND, 256), np.float32)
        v = bonds_full >= 0
        fb[np.nonzero(v)[0], :BFD] = f_bonds[bonds_full[v]]
        m = {"faT": np.ascontiguousarray(fa.T).astype(ml_dtypes.bfloat16),
             "fbT": np.ascontiguousarray(fb.T).astype(ml_dtypes.bfloat16)}
        for nm in ["snd", "aggW1", "aggW2", "rev", "g1", "fW1", "fW2", "f2r"]:
            m[nm] = d[nm]
        ins.append(m)
    return ins


def assemble_output(results):
    return np.concatenate([np.ascontiguousarray(r["mv_t"]).T for r in results],
                          axis=0)


def kernel(**inputs) -> np.ndarray:
    from concourse.bass_utils import run_bass_kernel_spmd
    if "nc" not in _CACHE:
        a2b = np.asarray(inputs["a2b"], np.int64)
        b2a = np.asarray(inputs["b2a"], np.int64)
        b2revb = np.asarray(inputs["b2revb"], np.int64)
        rt, S_pad, BT, NT, W2B, NT1, AT2, ktile = build_routing(a2b, b2a, b2revb)
        _CACHE["rt"] = (rt, S_pad, BT, NT, W2B, NT1, AT2, ktile)
        _CACHE["nc"] = build_kernel(S_pad, BT, NT, W2B, NT1, AT2, ktile)
    rt, S_pad, BT, NT, W2B, NT1, AT2, ktile = _CACHE["rt"]
    nc = _CACHE["nc"]
    wmap = prep_weights(inputs, BT)
    ins = prep_inputs(inputs, rt, BT)
    for m in ins:
        m.update(wmap)
    res = run_bass_kernel_spmd(nc, ins, core_ids=list(range(N_CORES)))
    return assemble_output(res.results).astype(np.float32)



# revision 19
# speedup vs baseline: 1.0376x; 1.0376x over previous
# CMPN encoder Bass kernel for 8-core TRN2, v2.
#
# Sharding: atoms by molecule (6400/core + atom0 replicated + pad = 6528);
# bonds reassigned so owner(b) = owner(atom b2a[b]) -> all b2a gathers local.
# Per round ONE AllToAll exchanges the bond-message rows each consumer needs
# (deduped union of its a2b and b2revb draws), delivered into T_union.
# Consumption uses int16 dma_gather with two 32768-row windows; slots whose
# source falls outside a window point at a zero row (messages are >= 0, so a
# zero row is an identity for both the sum and the max of the aggregation,
# and contributes nothing to the 2-part rev subtraction).
# Tables are bf16 with rows padded to 384 cols (zeros) so dma_gather's
# 256B-multiple elem_size holds and matmul K-chunks are exactly 3x128.
import numpy as np
import ml_dtypes
import concourse.bass as bass
import concourse.bacc as bacc
import concourse.tile as tile
from concourse import mybir
from concourse.library_config import mlp
from concourse.masks import make_identity

F32 = mybir.dt.float32
BF16 = mybir.dt.bfloat16
I16 = mybir.dt.int16
AF = mybir.ActivationFunctionType
ALU = mybir.AluOpType

H = 300
HP = 384
B = 1024
A = 50
NA = 51201
NB = 102401
AFD = 133
BFD = 147
N_CORES = 8
NMOL = B // N_CORES          # 128
ATOMS = NMOL * A             # 6400 real atoms/core
AT = 51                      # atom tiles: 6400 + atom0 + pad = 6528
DEPTH = 4
KCH = [(0, 128), (128, 256), (256, 300)]
WIN = 32768


# ======================= host-side routing prep ==========================

def _wrap_idx(vals):
    """int16 index list -> [128, ceil(n/16)] wrapped/replicated layout."""
    v = np.asarray(vals, np.int64)
    n = len(v)
    cols = -(-n // 16)
    arr = np.zeros((16, cols), np.int16)
    pad = np.zeros(cols * 16, np.int64)
    pad[:n] = v
    assert pad.max() < 32768 and pad.min() >= -32768
    arr = pad.reshape(cols, 16).T.astype(np.int16)
    return np.tile(arr, (8, 1))


def build_routing(a2b, b2a, b2revb):
    """All static routing. Returns per-core dicts + global sizes."""
    rt = {}
    owner_b = np.where(b2a >= 1, (b2a - 1) // 6400, np.arange(NB) % N_CORES)
    bonds = [np.where(owner_b == k)[0] for k in range(N_CORES)]
    nbond_max = max(len(x) for x in bonds)
    BT = -(-nbond_max // 128)
    NBND = BT * 128

    # consumer draw lists (atom agg for 6528-atom layout incl atom0 + pads)
    atom_ids = []
    for k in range(N_CORES):
        ids = np.concatenate([np.arange(6400 * k + 1, 6400 * (k + 1) + 1),
                              [0], np.zeros(127, np.int64)])
        atom_ids.append(ids)
    # agg slot draws: for pad atoms use bond draw "-1" => dummy
    agg_draws = []
    for k in range(N_CORES):
        d = np.full((AT * 128, 6), -1, np.int64)
        ids = atom_ids[k]
        real = np.concatenate([np.arange(6400), [6400]])  # local rows w/ draws
        d[real] = a2b[ids[real]]
        agg_draws.append(d)

    rev_draws = []
    for k in range(N_CORES):
        d = np.full(NBND, -1, np.int64)
        d[:len(bonds[k])] = b2revb[bonds[k]]
        rev_draws.append(d)

    # per-consumer union of needed bond rows, partitioned by owner
    I = [[None] * N_CORES for _ in range(N_CORES)]  # I[j][k]: global bond ids
    for k in range(N_CORES):
        u = np.unique(np.concatenate([agg_draws[k].ravel(), rev_draws[k]]))
        u = u[u >= 0]
        for j in range(N_CORES):
            I[j][k] = u[owner_b[u] == j]
    S = max(len(I[j][k]) for j in range(N_CORES) for k in range(N_CORES))
    S_pad = -(-S // 128) * 128
    NT = 2 + 8 * S_pad          # T_union rows: [z0 | recv 8*S_pad | z1]
    W2B = NT - WIN              # base row of window 2
    assert NT - 1 - W2B <= 32767 and WIN - 1 <= 32767

    # position of global bond id in consumer k's T_union (1-based after z0)
    pos = []
    for k in range(N_CORES):
        p = {}
        for j in range(N_CORES):
            for i, b in enumerate(I[j][k]):
                p[b] = 1 + j * S_pad + i
        pos.append(p)

    # bucket atoms by forced-W1 count f1 (slots with pos < W2B); per-f1
    # group padded to tiles of 128, group tile-counts equalized across cores.
    # Slot split per atom: the f1 forced-W1 slots gather via W1; the rest
    # (free or forced-W2) via W2 (valid: free slots have pos >= W2B).
    atom_f1 = []
    for k in range(N_CORES):
        f1 = np.zeros(6401, np.int64)
        slot_w1 = np.zeros((6401, 6), bool)
        for a in range(6401):
            for c in range(6):
                b = agg_draws[k][a, c]
                pp = pos[k][b] if b >= 0 else (WIN + 1)
                if pp < W2B:
                    slot_w1[a, c] = True
            f1[a] = slot_w1[a].sum()
        atom_f1.append((f1, slot_w1))
    grp_tiles = [max(-(-int((atom_f1[k][0] == f).sum()) // 128) or 0
                     for k in range(N_CORES)) for f in range(7)]
    # ensure nonzero handled: groups with zero atoms on all cores get 0 tiles
    for f in range(7):
        if all((atom_f1[k][0] == f).sum() == 0 for k in range(N_CORES)):
            grp_tiles[f] = 0
    AT2 = sum(grp_tiles)
    ktile = []
    for f in range(7):
        ktile += [f] * grp_tiles[f]
    # per-core permutation: new row -> old local atom (or -1 pad)
    perm_new2old = []
    perm_old2new = []
    for k in range(N_CORES):
        f1 = atom_f1[k][0]
        order = np.full(AT2 * 128, -1, np.int64)
        o2n = np.zeros(6401, np.int64)
        base = 0
        for f in range(7):
            rows = np.nonzero(f1 == f)[0]
            order[base:base + len(rows)] = rows
            o2n[rows] = base + np.arange(len(rows))
            base += grp_tiles[f] * 128
        perm_new2old.append(order)
        perm_old2new.append(o2n)

    # order bonds within each core: window-1 rev draws first, then window-2,
    # each group padded to a tile boundary. Recompute BT for group padding.
    bond_wflag = []
    new_bonds = []
    for k in range(N_CORES):
        bk = bonds[k]
        rd = b2revb[bk]
        p = np.array([pos[k][b] if b >= 0 else 0 for b in rd])
        isw2 = p >= WIN
        g1b, g2b = bk[~isw2], bk[isw2]
        n1t = -(-len(g1b) // 128)
        n2t = -(-len(g2b) // 128)
        new_bonds.append((g1b, g2b, n1t, n2t))
    NT1 = max(n1t for (_, _, n1t, n2t) in new_bonds)
    NT2 = max(n2t for (_, _, n1t, n2t) in new_bonds)
    BT = NT1 + NT2
    NBND = BT * 128
    bonds2 = []
    for k in range(N_CORES):
        g1b, g2b, _, _ = new_bonds[k]
        b_ord = np.full(NBND, -1, np.int64)
        b_ord[:len(g1b)] = g1b
        b_ord[NT1 * 128:NT1 * 128 + len(g2b)] = g2b
        wf = np.zeros(BT, np.int64)
        wf[NT1:] = 1
        bond_wflag.append(wf)
        bonds2.append(b_ord)
    # rebuild rev_draws in the new order
    rev_draws = []
    for k in range(N_CORES):
        d2 = np.full(NBND, -1, np.int64)
        valid = bonds2[k] >= 0
        d2[valid] = b2revb[bonds2[k][valid]]
        rev_draws.append(d2)
    bonds = [b[b >= 0] for b in bonds2]
    bonds_full = bonds2

    # local row of bond b on its owner core (position in new order)
    lrow = np.zeros(NB, np.int64)
    for k in range(N_CORES):
        v = bonds_full[k] >= 0
        lrow[bonds_full[k][v]] = np.nonzero(v)[0]

    out = []
    for k in range(N_CORES):
        d = {}
        # send list: for each dest kk, local rows of I[k][kk] padded to S_pad
        snd = np.zeros((N_CORES, S_pad), np.int64)
        for kk in range(N_CORES):
            ii = I[k][kk]
            snd[kk, :len(ii)] = lrow[ii]
        d["snd"] = _wrap_idx(snd.ravel())
        d["n_snd"] = N_CORES * S_pad

        def windows(draws_flat, pk):
            """draws (global bond ids, -1=dummy) -> (w1 idx, w2 idx) lists."""
            n = len(draws_flat)
            w1 = np.zeros(n, np.int64)
            w2 = np.full(n, 32767, np.int64)  # z1 relative pos = NT-1-W2B
            z1rel = NT - 1 - W2B
            w2[:] = z1rel
            for i, b in enumerate(draws_flat):
                if b < 0:
                    continue
                p = pk[b]
                if p < WIN:
                    w1[i] = p
                else:
                    w2[i] = p - W2B
            return w1, w2

        # atom agg, bucketed: tile with k=f: W1 part = k cols, W2 = 6-k cols
        f1, slot_w1 = atom_f1[k]
        aW1, aW2 = [], []
        z1rel = NT - 1 - W2B
        for ti in range(AT2):
            kk_ = ktile[ti]
            w1c = np.zeros((kk_, 128), np.int64)
            w2c = np.full((6 - kk_, 128), z1rel, np.int64)
            for p128 in range(128):
                old = perm_new2old[k][ti * 128 + p128]
                if old < 0:
                    w1c[:, p128] = 0
                    continue
                c1 = c2 = 0
                for c in range(6):
                    b = agg_draws[k][old, c]
                    pp = pos[k][b] if b >= 0 else None
                    if b >= 0 and pp < W2B:
                        w1c[c1, p128] = pp; c1 += 1
                    elif b >= 0:
                        w2c[c2, p128] = (pp - W2B); c2 += 1
                    else:
                        w2c[c2, p128] = z1rel; c2 += 1
            aW1.append(w1c.ravel()); aW2.append(w2c.ravel())
        d["aggW1"] = _wrap_idx(np.concatenate(aW1) if aW1 else np.zeros(16))
        d["aggW2"] = _wrap_idx(np.concatenate(aW2) if aW2 else np.zeros(16))
        # rev: bonds were ordered so each tile is window-pure; emit single
        # per-tile idx lists plus the per-tile window flag
        w1, w2 = windows(rev_draws[k], pos[k])
        wflag = bond_wflag[k]
        rev_single = np.where(np.repeat(wflag, 128) == 0, w1, w2)
        d["rev"] = _wrap_idx(rev_single)
        # g1: local atom row of b2a (transpose-mode, own-ma table)
        g1 = np.zeros(NBND, np.int64)
        v = bonds_full[k] >= 0
        bv = bonds_full[k][v]
        g1[np.nonzero(v)[0]] = perm_old2new[k][np.where(b2a[bv] >= 1, (b2a[bv] - 1) % 6400, 6400)]
        d["g1"] = _wrap_idx(g1)
        # F2 ma/ia rows: tile t -> local rows m*A+t (transpose-mode gathers)
        f2r = np.zeros((A, NMOL), np.int64)
        for t in range(A):
            f2r[t] = perm_old2new[k][np.arange(NMOL) * A + t]
        d["f2r"] = _wrap_idx(f2r.ravel())
        # F2 agg: atoms in (mol, t) tiles: tile t rows = atom local m*A+t
        fd = np.full((A, 128, 6), -1, np.int64)
        for t in range(A):
            rows = np.arange(NMOL) * A + t
            fd[t] = a2b[6400 * k + 1 + rows]
        fcol = fd.transpose(0, 2, 1).reshape(-1)
        w1, w2 = windows(fcol, pos[k])
        d["fW1"], d["fW2"] = _wrap_idx(w1), _wrap_idx(w2)
        d["bonds_full"] = bonds_full[k]
        d["perm"] = perm_new2old[k]
        out.append(d)
    return out, S_pad, BT, NT, W2B, NT1, AT2, ktile


# ======================= weights =========================================

def prep_weights(inp, BT):
    w = {kk: np.asarray(inp[kk], np.float32) for kk in
         ["W_i_atom", "W_i_bond", "W_h", "W_lr", "gru_bias", "w_ih_f", "w_hh_f",
          "b_ih_f", "b_hh_f", "w_ih_b", "w_hh_b", "b_ih_b", "b_hh_b", "W_o", "b_o"]}
    bf = lambda x: np.ascontiguousarray(x).astype(ml_dtypes.bfloat16)
    out = {}
    Wia = np.zeros((256, HP), np.float32); Wia[:AFD, :H] = w["W_i_atom"]
    Wib = np.zeros((256, HP), np.float32); Wib[:BFD, :H] = w["W_i_bond"]
    out["Wia"], out["Wib"] = bf(Wia), bf(Wib)
    for dd in range(DEPTH - 1):
        Wh = np.zeros((HP, HP), np.float32); Wh[:H, :H] = w["W_h"][dd]
        out[f"Wh{dd}"] = bf(Wh)
    for p in range(3):
        out[f"Wlr{p}"] = bf(w["W_lr"][p * H:(p + 1) * H])
    out["grub"] = bf(w["gru_bias"][None, :])
    for dd in "fb":
        out[f"wihT{dd}"] = bf(w[f"w_ih_{dd}"].T)
        out[f"whhT{dd}"] = bf(w[f"w_hh_{dd}"].T)
        out[f"bih{dd}"] = bf(w[f"b_ih_{dd}"][None, :])
        out[f"bhh{dd}"] = bf(w[f"b_hh_{dd}"][None, :])
    Wo_pad = np.zeros((6 * 128, H), np.float32)
    for i, (k0, k1) in enumerate(KCH):
        Wo_pad[i * 128:i * 128 + (k1 - k0)] = w["W_o"][k0:k1]
        Wo_pad[(3 + i) * 128:(3 + i) * 128 + (k1 - k0)] = w["W_o"][H + k0:H + k1]
    out["Wo"] = bf(Wo_pad)
    out["bo"] = bf(w["b_o"][None, :])
    # gi bias: b_ih for all gates + b_hh for r,z only (n's b_hh is applied
    # inside the recurrence, scaled by r)
    for dd in "fb":
        bc = w[f"b_ih_{dd}"].copy()
        bc[:2 * H] += w[f"b_hh_{dd}"][:2 * H]
        out[f"bgi{dd}"] = bf(bc[None, :])
        out[f"bhn{dd}"] = bf(w[f"b_hh_{dd}"][None, 2 * H:])
    return out


WSPEC = (
    [("Wia", (256, HP)), ("Wib", (256, HP))]
    + [(f"Wh{d}", (HP, HP)) for d in range(DEPTH - 1)]
    + [(f"Wlr{p}", (H, H)) for p in range(3)]
    + [("grub", (1, H))]
    + [(f"wihT{d}", (H, 3 * H)) for d in "fb"]
    + [(f"whhT{d}", (H, 3 * H)) for d in "fb"]
    + [(f"bgi{d}", (1, 3 * H)) for d in "fb"]
    + [(f"bhn{d}", (1, H)) for d in "fb"]
    + [("Wo", (6 * 128, H)), ("bo", (1, H))]
)


# ======================= kernel build ====================================

def build_kernel(S_pad, BT, NT, W2B, NT1, AT2, ktile, mp_iters=DEPTH - 1, do_final=True,
                 a2a_local=False):
    NBND = BT * 128
    nc = bacc.Bacc("TRN2", target_bir_lowering=False, debug=False,
                   num_devices=N_CORES)

    faT = nc.dram_tensor("faT", [256, AT2 * 128], BF16, kind="ExternalInput")
    fbT = nc.dram_tensor("fbT", [256, NBND], BF16, kind="ExternalInput")
    idx_specs = {
        "snd": 8 * S_pad,
        "aggW1": sum(ktile) * 128, "aggW2": (6 * AT2 - sum(ktile)) * 128,
        "rev": NBND, "g1": NBND,
        "fW1": A * 768, "fW2": A * 768, "f2r": A * NMOL,
    }
    idr = {nm: nc.dram_tensor(nm, [128, -(-n // 16)], I16, kind="ExternalInput")
           for nm, n in idx_specs.items()}
    wdr = {nm: nc.dram_tensor(nm, list(sh), BF16, kind="ExternalInput")
           for nm, sh in WSPEC}
    mv_t = nc.dram_tensor("mv_t", [NMOL, H], F32, kind="ExternalOutput")

    # internal DRAM
    TU = nc.dram_tensor("t_union", [NT, HP], BF16)
    SND = nc.dram_tensor("sendb", [8 * S_pad, HP], BF16)
    MB = nc.dram_tensor("own_mb", [NBND, HP], BF16)
    MAb = nc.dram_tensor("own_ma", [AT2 * 128, HP], BF16)
    AGG = nc.dram_tensor("agg_t", [AT2 * 128, HP], BF16)
    IA = nc.dram_tensor("ia_t", [AT2 * 128, HP], BF16)
    IB = nc.dram_tensor("ib_t", [NBND, HP], BF16)
    gi_d = {d: nc.dram_tensor(f"gi_{d}", [A, NMOL, 3 * H], BF16) for d in "fb"}
    hT_d = {d: nc.dram_tensor(f"hT_{d}", [A, 3, 128, NMOL], BF16) for d in "fb"}

    rg = [list(range(N_CORES))]

    with tile.TileContext(nc) as tc:
        with tc.tile_pool(name="const", bufs=1) as cp:
            nc.gpsimd.load_library(mlp)
            ident = cp.tile([128, 128], F32, tag="ident")
            make_identity(nc, ident[:])
            identb = cp.tile([128, 128], BF16, tag="identb")
            nc.vector.tensor_copy(out=identb[:], in_=ident[:])
            ones = cp.tile([1, 128], BF16, tag="ones")
            nc.vector.memset(ones[:], 1.0)
            zrow = cp.tile([2, HP], BF16, tag="zrow")
            nc.vector.memset(zrow[:], 0.0)
            nc.sync.dma_start(out=TU[0:1, :], in_=zrow[:1, :])
            nc.sync.dma_start(out=TU[NT - 1:NT, :], in_=zrow[1:2, :])

            idx = {}
            for nm in idx_specs:
                t = cp.tile(list(idr[nm].shape), I16, tag=f"ix_{nm}")
                nc.sync.dma_start(out=t[:], in_=idr[nm][:])
                idx[nm] = t

            def wchunks(nm, dt=BF16):
                dr = wdr[nm]
                outs = []
                for i, k0 in enumerate(range(0, dr.shape[0], 128)):
                    k1 = min(k0 + 128, dr.shape[0])
                    t = cp.tile([k1 - k0, dr.shape[1]], dt, tag=f"{nm}_{i}")
                    nc.sync.dma_start(out=t[:], in_=dr[k0:k1, :])
                    outs.append(t)
                return outs

            def load1(nm, dt=BF16):
                t = cp.tile(list(wdr[nm].shape), dt, tag=nm)
                nc.sync.dma_start(out=t[:], in_=wdr[nm][:])
                return t

            Wia = wchunks("Wia")
            Wib = wchunks("Wib")
            Wh = [wchunks(f"Wh{d}") for d in range(DEPTH - 1)]
            Wlr = [wchunks(f"Wlr{p}") for p in range(3)]
            grub = load1("grub")
            wihT = {d: wchunks(f"wihT{d}") for d in "fb"}
            whhT = {d: wchunks(f"whhT{d}") for d in "fb"}
            bgi = {d: load1(f"bgi{d}") for d in "fb"}
            bhn = {d: load1(f"bhn{d}") for d in "fb"}
            WoC = wchunks("Wo")
            bo = load1("bo")

            # ---------------- exchange helper -----------------------------
            def send_and_a2a(pool, src=MB):
                for blk in range(8):
                    g = pool.tile([128, S_pad // 128, HP], BF16, tag="sndg")
                    for p0 in range(0, S_pad, 1024):
                        n = min(1024, S_pad - p0)
                        nc.gpsimd.dma_gather(
                            g[:, p0 // 128:(p0 + n) // 128, :], src[:, :],
                            idx["snd"][:, (blk * S_pad + p0) // 16:
                                       (blk * S_pad + p0 + n) // 16],
                            n, n, HP)
                    nc.sync.dma_start(
                        out=SND.ap()[blk * S_pad:(blk + 1) * S_pad, :]
                            .rearrange("(a p) h -> p a h", p=128),
                        in_=g[:])
                if a2a_local:
                    nc.sync.dma_start(out=TU[1:1 + 8 * S_pad, :], in_=SND[:, :])
                else:
                    nc.gpsimd.collective_compute(
                        "AllToAll", ALU.bypass, replica_groups=rg,
                        ins=[SND[:, :]], outs=[TU[1:1 + 8 * S_pad, :]])

            # ---------------- stage 0: input projections -----------------
            def proj(pool, psum, src_t, Wch, t0, bt, outs_bf):
                """project bt tiles starting at t0; outs_bf: list of dram"""
                lhs = pool.tile([128, 2, bt * 128], BF16, tag="plhs")
                nc.sync.dma_start(
                    out=lhs[:],
                    in_=src_t.ap()[:, t0 * 128:(t0 + bt) * 128]
                        .rearrange("(c p) n -> p c n", p=128))
                stg = pool.tile([128, bt, HP], BF16, tag="pstg")
                for i in range(bt):
                    pm = psum.tile([128, HP], F32, tag="pmm")
                    for c in range(2):
                        nc.tensor.matmul(out=pm[:], lhsT=lhs[:, c, i * 128:(i + 1) * 128],
                                         rhs=Wch[c][:], start=(c == 0), stop=(c == 1),
                                         skip_group_check=True)
                    nc.scalar.activation(out=stg[:, i, :], in_=pm[:], func=AF.Relu)
                for dr in outs_bf:
                    nc.sync.dma_start(
                        out=dr.ap()[(t0) * 128:(t0 + bt) * 128, :]
                            .rearrange("(a p) h -> p a h", p=128),
                        in_=stg[:])

            PB = 4
            with (tc.tile_pool(name="s0", bufs=3) as pool,
                  tc.tile_pool(name="s0p", bufs=4, space="PSUM") as psum,
                  tc.tile_pool(name="snd0e", bufs=2) as sndp):
                for t0 in range(0, BT, PB):
                    bt = min(PB, BT - t0)
                    proj(pool, psum, fbT, Wib, t0, bt, [IB])
                send_and_a2a(sndp, src=IB)
                for t0 in range(0, AT2, PB):
                    bt = min(PB, AT2 - t0)
                    proj(pool, psum, faT, Wia, t0, bt, [MAb, IA])

            w1rows = min(NT, WIN)
            w2rows = NT - W2B

            # ---------------- message passing rounds ---------------------
            AB = 3    # atom tiles per batch
            BB = 4    # bond tiles per batch
            kcum1 = [0]
            kcum2 = [0]
            for f in ktile:
                kcum1.append(kcum1[-1] + f * 8)       # idx cols (128 idx = 8)
                kcum2.append(kcum2[-1] + (6 - f) * 8)
            for r in range(mp_iters):
                with (tc.tile_pool(name=f"A{r}", bufs=2) as pool,):
                    for t0 in range(0, AT2, AB):
                        bt = min(AB, AT2 - t0)
                        g1 = pool.tile([128, bt * 6, HP], BF16, tag="G1")
                        g2 = pool.tile([128, bt * 6, HP], BF16, tag="G2")
                        for i in range(bt):
                            kk_ = ktile[t0 + i]
                            if kk_ > 0:
                                nc.gpsimd.dma_gather(
                                    g1[:, i * 6:i * 6 + kk_, :], TU[0:w1rows, :],
                                    idx["aggW1"][:, kcum1[t0 + i]:kcum1[t0 + i + 1]],
                                    kk_ * 128, kk_ * 128, HP)
                            if kk_ < 6:
                                nc.gpsimd.dma_gather(
                                    g2[:, i * 6:i * 6 + 6 - kk_, :], TU[W2B:NT, :],
                                    idx["aggW2"][:, kcum2[t0 + i]:kcum2[t0 + i + 1]],
                                    (6 - kk_) * 128, (6 - kk_) * 128, HP)
                        mab = pool.tile([128, bt, HP], BF16, tag="mab")
                        nc.sync.dma_start(
                            in_=MAb.ap()[t0 * 128:(t0 + bt) * 128, :]
                                .rearrange("(a p) h -> p a h", p=128),
                            out=mab[:])
                        for i in range(bt):
                            kk_ = ktile[t0 + i]
                            ops = [g1[:, i * 6 + c, :H] for c in range(kk_)] +                                   [g2[:, i * 6 + c, :H] for c in range(6 - kk_)]
                            ssum = pool.tile([128, H], BF16, tag="ssum")
                            nc.vector.tensor_add(out=ssum[:], in0=ops[0], in1=ops[1])
                            smax = pool.tile([128, H], BF16, tag="smax")
                            nc.vector.tensor_tensor(out=smax[:], in0=ops[0],
                                                    in1=ops[1], op=ALU.max)
                            for c in range(2, 6):
                                nc.vector.tensor_add(out=ssum[:], in0=ssum[:],
                                                     in1=ops[c])
                                nc.vector.tensor_tensor(out=smax[:], in0=smax[:],
                                                        in1=ops[c], op=ALU.max)
                            nc.vector.tensor_mul(out=ssum[:], in0=ssum[:], in1=smax[:])
                            nc.vector.tensor_add(out=mab[:, i, :H],
                                                 in0=mab[:, i, :H],
                                                 in1=ssum[:])
                        nc.sync.dma_start(
                            out=MAb.ap()[t0 * 128:(t0 + bt) * 128, :]
                                .rearrange("(a p) h -> p a h", p=128),
                            in_=mab[:])

                with (tc.tile_pool(name=f"B{r}", bufs=3) as pool,
                      tc.tile_pool(name=f"B{r}p", bufs=4, space="PSUM") as psum):
                    bstarts = []
                    for t0 in range(0, NT1, BB):
                        bstarts.append((t0, min(BB, NT1 - t0)))
                    for t0 in range(NT1, BT, BB):
                        bstarts.append((t0, min(BB, BT - t0)))
                    for t0, bt in bstarts:
                        n = bt * 128
                        g1t = pool.tile([128, 3, n], BF16, tag="g1t")
                        nc.gpsimd.dma_gather(
                            g1t[:], MAb[:, :],
                            idx["g1"][:, t0 * 8:t0 * 8 + n // 16], n, n, HP,
                            transpose=True)
                        g2a = pool.tile([128, 3, n], BF16, tag="g2a")
                        src = TU[0:w1rows, :] if t0 < NT1 else TU[W2B:NT, :]
                        nc.gpsimd.dma_gather(
                            g2a[:], src,
                            idx["rev"][:, t0 * 8:t0 * 8 + n // 16], n, n, HP,
                            transpose=True)
                        nc.vector.tensor_sub(out=g1t[:], in0=g1t[:], in1=g2a[:])
                        ibt = pool.tile([128, bt, HP], BF16, tag="ibt")
                        nc.sync.dma_start(
                            out=ibt[:],
                            in_=IB.ap()[t0 * 128:(t0 + bt) * 128, :]
                                .rearrange("(a p) h -> p a h", p=128))
                        stg = pool.tile([128, bt, HP], BF16, tag="mstg")
                        for i in range(bt):
                            pm = psum.tile([128, HP], F32, tag="bmm")
                            for c in range(3):
                                nc.tensor.matmul(
                                    out=pm[:], lhsT=g1t[:, c, i * 128:(i + 1) * 128],
                                    rhs=Wh[r][c][:], start=(c == 0), stop=False,
                                    skip_group_check=True)
                            nc.tensor.matmul(
                                out=pm[:], lhsT=identb[:], rhs=ibt[:, i, :],
                                start=False, stop=True, skip_group_check=True)
                            nc.scalar.activation(out=stg[:, i, :], in_=pm[:],
                                                 func=AF.Relu)
                        nc.sync.dma_start(
                            out=MB.ap()[t0 * 128:(t0 + bt) * 128, :]
                                .rearrange("(a p) h -> p a h", p=128),
                            in_=stg[:])

                with tc.tile_pool(name=f"X{r}", bufs=2) as pool:
                    send_and_a2a(pool)

            if not do_final:
                nc.compile()
                return nc

            # ---------------- final readout + GRU + output ----------------
            def transpose_chunks(pool, psum, src, dtype=BF16, eng="act", tag="tx"):
                outs = []
                for ci, (k0, k1) in enumerate(KCH):
                    pt = psum.tile([128, 128], src.dtype, tag="tp")
                    idt = ident if src.dtype == F32 else identb
                    nc.tensor.transpose(out=pt[:k1 - k0, :], in_=src[:, k0:k1],
                                        identity=idt[:])
                    st = pool.tile([128, 128], dtype, tag=f"{tag}{ci}")
                    if eng == "act":
                        nc.scalar.copy(out=st[:k1 - k0, :], in_=pt[:k1 - k0, :])
                    else:
                        nc.vector.tensor_copy(out=st[:k1 - k0, :], in_=pt[:k1 - k0, :])
                    outs.append(st)
                return outs

            def mm_kchunks(pt, lhsT_tiles, rhs_tiles, start=True, stop=True,
                           n0=0, n1=H):
                for ci, (lt, rt) in enumerate(zip(lhsT_tiles, rhs_tiles)):
                    klen = min(lt.shape[0], rt.shape[0])
                    nc.tensor.matmul(out=pt[:], lhsT=lt[:klen, :],
                                     rhs=rt[:klen, n0:n1],
                                     start=(start and ci == 0),
                                     stop=(stop and ci == len(lhsT_tiles) - 1),
                                     skip_group_check=True)

            with tc.tile_pool(name="h0p", bufs=1) as h0p:
                h0 = h0p.tile([128, H], F32, tag="h0")
                nc.vector.memset(h0[:], -1e30)
                FB = 3
                with (tc.tile_pool(name="F", bufs=2) as pool,
                      tc.tile_pool(name="Fp", bufs=2, space="PSUM") as psum,
                      tc.tile_pool(name="Fg", bufs=2, space="PSUM") as psg):
                    for t0 in range(0, A, FB):
                        bt = min(FB, A - t0)
                        g1 = pool.tile([128, bt * 6, HP], BF16, tag="fG1")
                        g2 = pool.tile([128, bt * 6, HP], BF16, tag="fG2")
                        for i in range(bt):
                            c0 = (t0 + i) * 48
                            nc.gpsimd.dma_gather(
                                g1[:, i * 6:(i + 1) * 6, :], TU[0:w1rows, :],
                                idx["fW1"][:, c0:c0 + 48], 768, 768, HP)
                            nc.gpsimd.dma_gather(
                                g2[:, i * 6:(i + 1) * 6, :], TU[W2B:NT, :],
                                idx["fW2"][:, c0:c0 + 48], 768, 768, HP)
                        for i in range(bt):
                            t = t0 + i
                            G1 = g1[:, i * 6:(i + 1) * 6, :]
                            G2 = g2[:, i * 6:(i + 1) * 6, :]
                            s3 = pool.tile([128, 3, H], BF16, tag="fs3")
                            nc.vector.tensor_add(out=s3[:], in0=G1[:, 0:3, :H],
                                                 in1=G1[:, 3:6, :H])
                            t3 = pool.tile([128, 3, H], BF16, tag="ft3")
                            nc.vector.tensor_add(out=t3[:], in0=G2[:, 0:3, :H],
                                                 in1=G2[:, 3:6, :H])
                            nc.vector.tensor_add(out=s3[:], in0=s3[:], in1=t3[:])
                            ssum = pool.tile([128, H], F32, tag="fsum")
                            nc.vector.tensor_add(out=ssum[:], in0=s3[:, 0, :],
                                                 in1=s3[:, 1, :])
                            nc.vector.tensor_add(out=ssum[:], in0=ssum[:],
                                                 in1=s3[:, 2, :])
                            m3 = pool.tile([128, 3, H], BF16, tag="fm3")
                            nc.vector.tensor_tensor(out=m3[:], in0=G1[:, 0:3, :H],
                                                    in1=G1[:, 3:6, :H], op=ALU.max)
                            m32 = pool.tile([128, 3, H], BF16, tag="fm32")
                            nc.vector.tensor_tensor(out=m32[:], in0=G2[:, 0:3, :H],
                                                    in1=G2[:, 3:6, :H], op=ALU.max)
                            nc.vector.tensor_tensor(out=m3[:], in0=m3[:], in1=m32[:],
                                                    op=ALU.max)
                            smax = pool.tile([128, H], F32, tag="fmax")
                            nc.vector.tensor_tensor(out=smax[:], in0=m3[:, 0, :],
                                                    in1=m3[:, 1, :], op=ALU.max)
                            nc.vector.tensor_tensor(out=smax[:], in0=smax[:],
                                                    in1=m3[:, 2, :], op=ALU.max)
                            agg = pool.tile([128, H], BF16, tag="fagg")
                            nc.vector.tensor_mul(out=agg[:], in0=ssum[:], in1=smax[:])
                            lhs_a = transpose_chunks(pool, psum, agg, tag="ta")
                            maT = pool.tile([128, 3, NMOL], BF16, tag="fmaT")
                            nc.gpsimd.dma_gather(
                                maT[:], MAb[:, :],
                                idx["f2r"][:, t * 8:(t + 1) * 8], NMOL, NMOL, HP,
                                transpose=True)
                            iaT = pool.tile([128, 3, NMOL], BF16, tag="fiaT")
                            nc.gpsimd.dma_gather(
                                iaT[:], IA[:, :],
                                idx["f2r"][:, t * 8:(t + 1) * 8], NMOL, NMOL, HP,
                                transpose=True)
                            pm = psum.tile([128, H], F32, tag="fmm")
                            mm_kchunks(pm, lhs_a, Wlr[0], start=True, stop=False)
                            for ci, (k0, k1) in enumerate(KCH):
                                nc.tensor.matmul(
                                    out=pm[:], lhsT=maT[:k1 - k0, ci, :],
                                    rhs=Wlr[1][ci][:k1 - k0, :],
                                    start=False, stop=False,
                                    skip_group_check=True)
                            for ci, (k0, k1) in enumerate(KCH):
                                nc.tensor.matmul(
                                    out=pm[:], lhsT=iaT[:k1 - k0, ci, :],
                                    rhs=Wlr[2][ci][:k1 - k0, :],
                                    start=False, stop=(ci == 2),
                                    skip_group_check=True)
                            nc.vector.tensor_tensor(out=h0[:], in0=h0[:], in1=pm[:],
                                                    op=ALU.max)
                            nc.tensor.matmul(out=pm[:], lhsT=ones[:, :], rhs=grub[:, :],
                                             start=False, stop=True,
                                             skip_group_check=True)
                            msg = pool.tile([128, H], BF16, tag="fmsg")
                            nc.scalar.activation(out=msg[:], in_=pm[:], func=AF.Relu)
                            lhs_x = transpose_chunks(pool, psum, msg, tag="txx")
                            for di in "fb":
                                gts = pool.tile([128, 3 * H], BF16, tag="fgts")
                                for half in range(2):
                                    n0, n1 = half * 450, (half + 1) * 450
                                    pg = psg.tile([128, 450], F32, tag="fgi")
                                    for ci, (k0, k1) in enumerate(KCH):
                                        nc.tensor.matmul(
                                            out=pg[:], lhsT=lhs_x[ci][:k1 - k0, :],
                                            rhs=wihT[di][ci][:k1 - k0, n0:n1],
                                            start=(ci == 0), stop=False,
                                            skip_group_check=True)
                                    nc.tensor.matmul(
                                        out=pg[:], lhsT=ones[:, :],
                                        rhs=bgi[di][:, n0:n1],
                                        start=False, stop=True,
                                        skip_group_check=True)
                                    nc.scalar.copy(out=gts[:, n0:n1], in_=pg[:])
                                nc.sync.dma_start(out=gi_d[di][t, :, :], in_=gts[:])

                # ---- GRU ---------------------------------------------------
                with (tc.tile_pool(name="gruh", bufs=2) as hp,
                      tc.tile_pool(name="gru", bufs=3) as gp,
                      tc.tile_pool(name="grut", bufs=2) as gtp,
                      tc.tile_pool(name="grutp", bufs=2, space="PSUM") as ptp,
                      tc.tile_pool(name="grup", bufs=2, space="PSUM") as pgh):
                    def hT_one(hsrc, tag):
                        big = gtp.tile([128, 3, 128], BF16, tag=tag)
                        cpy = [nc.scalar.copy, nc.vector.tensor_copy,
                               nc.scalar.copy]
                        for ci, (k0, k1) in enumerate(KCH):
                            pt = ptp.tile([128, 128], F32, tag="gtp")
                            nc.tensor.transpose(out=pt[:k1 - k0, :],
                                                in_=hsrc[:, k0:k1],
                                                identity=ident[:])
                            cpy[ci](out=big[:k1 - k0, ci, :],
                                    in_=pt[:k1 - k0, :])
                        return big
                    hcur, hT = {}, {}
                    for di in "fb":
                        ht = hp.tile([128, H], F32, tag=f"h_{di}")
                        nc.vector.tensor_copy(out=ht[:], in_=h0[:])
                        hcur[di] = ht
                        hT[di] = hT_one(ht, f"hx{di}")
                    for step in range(A):
                        for di in "fb":
                            t = step if di == "f" else A - 1 - step
                            h = hcur[di]
                            giw = gp.tile([128, 3 * H], BF16, tag="gil")
                            nc.sync.dma_start(out=giw[:], in_=gi_d[di][t, :, :])
                            gi = [giw[:, g * H:(g + 1) * H] for g in range(3)]
                            gh = []
                            for g in range(3):
                                pg = pgh.tile([128, H], F32, tag=f"gh{g}")
                                for ci, (k0, k1) in enumerate(KCH):
                                    nc.tensor.matmul(
                                        out=pg[:], lhsT=hT[di][:k1 - k0, ci, :],
                                        rhs=whhT[di][ci][:k1 - k0, g * H:(g + 1) * H],
                                        start=(ci == 0),
                                        stop=(g < 2 and ci == 2),
                                        skip_group_check=True)
                                gh.append(pg)
                            nc.tensor.matmul(
                                out=gh[2][:], lhsT=ones[:, :], rhs=bhn[di][:, :],
                                start=False, stop=True, skip_group_check=True)
                            rz = gp.tile([128, 2 * H], F32, tag="rz")
                            nc.vector.tensor_add(out=rz[:, 0:H], in0=gi[0],
                                                 in1=gh[0][:])
                            nc.vector.tensor_add(out=rz[:, H:2 * H], in0=gi[1],
                                                 in1=gh[1][:])
                            nc.scalar.activation(out=rz[:], in_=rz[:], func=AF.Sigmoid)
                            r_ = rz[:, 0:H]
                            z = rz[:, H:2 * H]
                            n_ = gp.tile([128, H], F32, tag="n")
                            nc.vector.tensor_mul(out=n_[:], in0=r_, in1=gh[2][:])
                            nc.vector.tensor_add(out=n_[:], in0=n_[:], in1=gi[2])
                            nc.scalar.activation(out=n_[:], in_=n_[:], func=AF.Tanh)
                            hn = hp.tile([128, H], F32, tag=f"h_{di}")
                            nc.vector.tensor_sub(out=hn[:], in0=h[:], in1=n_[:])
                            nc.vector.tensor_mul(out=hn[:], in0=z, in1=hn[:])
                            nc.vector.tensor_add(out=hn[:], in0=hn[:], in1=n_[:])
                            hcur[di] = hn
                            hT[di] = hT_one(hn, f"hx{di}")
                            nc.sync.dma_start(
                                out=hT_d[di].ap()[t, :, :, :]
                                    .rearrange("c p n -> p c n"),
                                in_=hT[di][:])

                # ---- F4: output projection + mol mean (mol-major) -----------
                with (tc.tile_pool(name="F4", bufs=3) as pool4,
                      tc.tile_pool(name="F4a", bufs=1, space="PSUM") as accp,
                      tc.tile_pool(name="F4p", bufs=2, space="PSUM") as psum4):
                    mvacc = accp.tile([128, H], F32, tag="mvacc")
                    for t in range(A):
                        cts = []
                        for di in "fb":
                            ct = pool4.tile([128, 3, NMOL], BF16, tag=f"c{di}")
                            nc.sync.dma_start(
                                out=ct[:],
                                in_=hT_d[di].ap()[t, :, :, :]
                                    .rearrange("c p n -> p c n"))
                            cts.append(ct)
                        pm = psum4.tile([128, H], F32, tag="f4mm")
                        for ii in range(6):
                            k0, k1 = KCH[ii % 3]
                            nc.tensor.matmul(
                                out=pm[:],
                                lhsT=cts[ii // 3][:k1 - k0, ii % 3, :],
                                rhs=WoC[ii][:k1 - k0, :],
                                start=(ii == 0), stop=False,
                                skip_group_check=True)
                        nc.tensor.matmul(out=pm[:], lhsT=ones[:, :], rhs=bo[:, :],
                                         start=False, stop=True,
                                         skip_group_check=True)
                        ah = pool4.tile([128, H], BF16, tag="ah")
                        nc.scalar.activation(out=ah[:], in_=pm[:], func=AF.Relu)
                        nc.tensor.matmul(out=mvacc[:], lhsT=identb[:], rhs=ah[:],
                                         start=(t == 0), stop=(t == A - 1),
                                         skip_group_check=True)
                    mvf = pool4.tile([128, H], F32, tag="mvf")
                    nc.scalar.mul(out=mvf[:], in_=mvacc[:], mul=1.0 / A)
                    nc.sync.dma_start(out=mv_t[:, :], in_=mvf[:])

    nc.compile()
    return nc


# ======================= host entry ======================================

_CACHE = {}


def prep_inputs(inp, rt, BT):
    f_atoms = np.asarray(inp["f_atoms"], np.float32)
    f_bonds = np.asarray(inp["f_bonds"], np.float32)
    NBND = BT * 128
    ins = []
    for k in range(N_CORES):
        d = dict(rt[k])
        bonds_full = d.pop("bonds_full")
        perm = d.pop("perm")
        AT2 = len(perm) // 128
        old_ids = np.concatenate([np.arange(6400 * k + 1, 6400 * (k + 1) + 1),
                                  [0]])
        fa = np.zeros((AT2 * 128, 256), np.float32)
        vv = perm >= 0
        fa[np.nonzero(vv)[0], :AFD] = f_atoms[old_ids[perm[vv]]]
        fb = np.zeros((NBND, 256), np.float32)
        v = bonds_full >= 0
        fb[np.nonzero(v)[0], :BFD] = f_bonds[bonds_full[v]]
        m = {"faT": np.ascontiguousarray(fa.T).astype(ml_dtypes.bfloat16),
             "fbT": np.ascontiguousarray(fb.T).astype(ml_dtypes.bfloat16)}
        for nm in ["snd", "aggW1", "aggW2", "rev", "g1", "fW1", "fW2", "f2r"]:
            m[nm] = d[nm]
        ins.append(m)
    return ins


def assemble_output(results):
    return np.concatenate([np.ascontiguousarray(r["mv_t"]) for r in results],
                          axis=0)


def kernel(**inputs) -> np.ndarray:
    from concourse.bass_utils import run_bass_kernel_spmd
    if "nc" not in _CACHE:
        a2b = np.asarray(inputs["a2b"], np.int64)
        b2a = np.asarray(inputs["b2a"], np.int64)
        b2revb = np.asarray(inputs["b2revb"], np.int64)
        rt, S_pad, BT, NT, W2B, NT1, AT2, ktile = build_routing(a2b, b2a, b2revb)
        _CACHE["rt"] = (rt, S_pad, BT, NT, W2B, NT1, AT2, ktile)
        _CACHE["nc"] = build_kernel(S_pad, BT, NT, W2B, NT1, AT2, ktile)
    rt, S_pad, BT, NT, W2B, NT1, AT2, ktile = _CACHE["rt"]
    nc = _CACHE["nc"]
    wmap = prep_weights(inputs, BT)
    ins = prep_inputs(inputs, rt, BT)
    for m in ins:
        m.update(wmap)
    res = run_bass_kernel_spmd(nc, ins, core_ids=list(range(N_CORES)))
    return assemble_output(res.results).astype(np.float32)

